# revision 5
# baseline (speedup 1.0000x reference)
"""Gaussian RBF kernel for Trainium2, data-parallel over batch across 8 cores.

exp(-0.5*||x-mu||^2/sigma^2) folded into ONE augmented GEMM + Exp:
  E[s,o] = sum_d x[s,d]*(2*a[o]*mus[o,d]) + x2[s]*(-a[o]) + 1*(-a[o]*m2[o])
with a = 0.5/sigma^2.  Augmented contraction K = D+2 = 66; the tiny weight
matrix W (66,512) and the x2/ones augmentation are built on host and packed
with xT into one DRAM input buffer.

Per core: 32 matmuls (128,66)@(66,512) in float32r (1 cyc/row at free>=256,
4x faster than plain fp32), Exp on ACT engine in 4-tile chunks (128,2048)
PSUM->SBUF writing bf16, then one contiguous 512KB DMA per chunk.

The s-rows are permuted host-side so that SBUF partition p / tile t maps to
DRAM row 4p+t within each chunk: the output DMA is then fully contiguous
(4KB per partition) and the result comes back already row-major; bf16 is
upcast to fp32 on host (0.4% worst-case quantization, well inside 2e-2).

Raw bass engine programs (explicit semaphores) -- the Tile framework's
attached-wait sync scheme trips "Too many sync wait commands" in this
compiler build, so engines are programmed directly.
"""
import numpy as np
from concourse import bass, mybir
from concourse import bass_utils

B, S, D, O = 8, 4096, 64, 512
K = D + 2            # 66: [x, x2, 1]
P = 128              # rows (s) per matmul tile
TPC = 4              # tiles per chunk (ACT + DMA granularity)
CH = S // (P * TPC)  # 8 chunks
CW = TPC * O         # 2048 psum/sbuf cols per chunk
KW = O + S           # packed input cols: [W | x chunks 0..7]

FP = mybir.dt.float32
FR = mybir.dt.float32r
BF = mybir.dt.bfloat16


def _build():
    nc = bass.Bass()
    xaw = nc.declare_dram_parameter("xaw", [K, KW], FR, isOutput=False)
    out = nc.declare_dram_parameter("out", [CH, P, CW], BF, isOutput=True)

    with (
        nc.sbuf_tensor([K, KW], FR) as xt,
        nc.sbuf_tensor([P, CH * CW], BF) as ot,
        nc.psum_tensor([P, 2 * CW], FP) as ps,
        nc.Block() as block,
        nc.semaphore("dma_in") as dma_in,
        nc.semaphore("mm") as mm,
        nc.semaphore("act_s") as act_s,
        nc.semaphore("dma_out") as dma_out,
    ):
        xr = xt

        @block.sync
        def _(sync):
            # W + chunk 0, then chunks 1..7
            sync.dma_start(out=xt[:, : O + P * TPC],
                           in_=xaw[:, : O + P * TPC]).then_inc(dma_in, 16)
            sync.dma_start(out=xt[:, O + P * TPC:],
                           in_=xaw[:, O + P * TPC:]).then_inc(dma_in, 16)
            for c in range(CH):
                sync.wait_ge(act_s, c + 1)
                sync.dma_start(
                    out=out[c],
                    in_=ot[:, c * CW:(c + 1) * CW],
                ).then_inc(dma_out, 16)
            sync.wait_ge(dma_out, 16 * CH)

        @block.tensor
        def _(pe):
            pe.wait_ge(dma_in, 16)
            for c in range(CH):
                if c == 1:
                    pe.wait_ge(dma_in, 32)
                if c >= 2:
                    pe.wait_ge(act_s, c - 1)
                for t in range(TPC):
                    m = pe.matmul(
                        ps[:, (c % 2) * CW + t * O:(c % 2) * CW + (t + 1) * O],
                        xr[:, O + c * P * TPC + t * P: O + c * P * TPC + (t + 1) * P],
                        xr[:, :O],
                        start=True,
                        stop=True,
                    )
                    if t == TPC - 1:
                        m.then_inc(mm, 1)

        @block.scalar
        def _(scalar):
            for c in range(CH):
                scalar.wait_ge(mm, c + 1)
                scalar.activation(
                    ot[:, c * CW:(c + 1) * CW],
                    ps[:, (c % 2) * CW:(c % 2 + 1) * CW],
                    mybir.ActivationFunctionType.Exp,
                ).then_inc(act_s, 1)

    return nc


def kernel(x, mus, log_sigmas):
    x = np.asarray(x, np.float32)
    mus = np.asarray(mus, np.float32)
    log_sigmas = np.asarray(log_sigmas, np.float32)

    a = 0.5 * np.exp(-2.0 * log_sigmas.astype(np.float64))          # (O,)
    m2 = np.sum(mus.astype(np.float64) ** 2, axis=1)                # (O,)
    W = np.empty((K, O), np.float32)
    W[:D] = (2.0 * a[None, :] * mus.T.astype(np.float64)).astype(np.float32)
    W[D] = (-a).astype(np.float32)
    W[D + 1] = (-a * m2).astype(np.float32)

    x2 = np.sum(x * x, axis=-1)                                     # (B,S)
    in_maps = []
    for i in range(B):
        xa = np.empty((S, K), np.float32)
        xa[:, :D] = x[i]
        xa[:, D] = x2[i]
        xa[:, D + 1] = 1.0
        # permute s so partition p / tile t <-> row 4p+t inside each chunk:
        # (c,p,t,K) -> (K, c, t, p) flattened to (K, S)
        xp = xa.reshape(CH, P, TPC, K).transpose(3, 0, 2, 1).reshape(K, S)
        xaw = np.empty((K, KW), np.float32)
        xaw[:, :O] = W
        xaw[:, O:] = xp
        in_maps.append({"xaw": xaw})

    nc = _build()
    res = bass_utils.run_bass_kernel_spmd(nc, in_maps, list(range(B)))
    global _last_results
    _last_results = res
    full = np.stack(
        [np.asarray(r["out"]).reshape(S, O) for r in res.results], axis=0
    )
    return full.astype(np.float32)


_last_results = None
